# revision 16
# baseline (speedup 1.0000x reference)
"""Trainium2 Bass kernel for DocumentClassificationGNN (3-layer GCN + BN/ReLU +
global mean pool + MLP head), distributed over 8 NeuronCores.

Strategy (node/graph parallel):
  - Nodes are assigned to (core, slot) sorted by in-degree so every core/tile
    carries a balanced edge load.  Edges are partitioned by DESTINATION core so
    the segment-sum scatter is device-local.
  - The symmetric edge norm deg^-1/2[src]*deg^-1/2[dst] is FACTORIZED:
    the src factor is folded into the feature table rows (scaled on write),
    the dst factor into the per-tile consume scale (launch B/C) or into the
    host-precomputed pooling matrix (launch D).  This makes the scatter
    routing matrices pure 0/1, which lets one DVE tensor_tensor(is_equal)
    build K=16 of them at once in 2x perf mode (j-major layout, the per-chunk
    [128,128] one-hot is a stride-K column slice used directly as matmul lhsT).
  - Per layer: each core dma_gathers its in-edge source rows from the
    host-replicated fp16 table and scatter-adds them into PSUM with the 0/1
    one-hot matmuls; consume = Act copy (x dinv_dst) -> PE transpose ->
    Act BN+ReLU (folded scale/bias) -> per-tile GEMM with W as rhs
    (out = [slot, fout] directly, no transpose back) -> Act copy (x dinv_src)
    -> per-tile DMA to the table shard.
  - Launch D accumulates onehot(batch)-weighted pooling matmuls into a single
    persistent PSUM bank across all 49 tiles.
  - Host: assemble/replicate the table between launches, fold BN params,
    final tiny classifier MLP in fp64.

Programs (3 compiles, 4 launches):
  A : T1 = (dinv*x) @ W1
  BC: Y = scatter(T); h = relu(bnfold(dinv_dst*Y)); T' = dinv_src*(h @ W)
  D : Y3 = scatter(T3); pooled_partial = poolmat^T @ Y3
"""

import hashlib
import numpy as np
from contextlib import ExitStack

import concourse.bass as bass
import concourse.bacc as bacc
import concourse.tile as tile
from concourse import mybir
from concourse.bass_utils import run_bass_kernel_spmd
from concourse.masks import make_identity

P = 128
NCORES = 8
N = 50000
D_IN = 256
H = 128
NGRAPH = 64
SLOTS = 6272            # 49 tiles of 128 slots per core (6250 real nodes + pad)
TILES = SLOTS // P      # 49
RAW = NCORES * SLOTS    # 50176
LOB = 32767             # table row 32767 is the lo-region zero row
TAB = RAW + 2           # +2 zero rows (lo @32767, hi @TAB-1)
ZLO = LOB               # lo-local zero row index
ZHI_LOCAL = TAB - 1 - 32768   # hi-local zero row index
# dst tiles per gather group: small leading groups shorten the pipeline ramp
GROUP_SIZES = [2, 3, 4, 5, 7, 7, 7, 7, 7]
NGROUPS = len(GROUP_SIZES)
GROUP_T0 = [sum(GROUP_SIZES[:i]) for i in range(NGROUPS)]
assert sum(GROUP_SIZES) == TILES
KB = 16                 # chunks per batched one-hot build
MAXCH = 8               # 1024 indices per dma_gather call (HW SWDGE ring limit)
BN_EPS = 1e-5

F16 = mybir.dt.float16
F32 = mybir.dt.float32
I16 = mybir.dt.int16

# module-level knobs / perf results (test.py pokes these)
TRACE = False
LAST_EXEC_NS = []       # per-launch exec_time_ns (when TRACE)

_PLAN_CACHE = {}
_PROG_CACHE = {}


# ---------------------------------------------------------------- host prep --

def _wrap_idx(flat):
    """dma_gather index layout: idx i -> [i%16, i//16], replicated to 128 parts."""
    n = len(flat)
    assert n % 16 == 0
    arr = np.asarray(flat, dtype=np.int16).reshape(n // 16, 16).T.copy()
    return np.tile(arr, (8, 1))


class _Plan:
    pass


def _make_plan(edge_index, batch, x):
    pl = _Plan()
    src = np.asarray(edge_index[0], dtype=np.int64)
    dst = np.asarray(edge_index[1], dtype=np.int64)
    batch = np.asarray(batch, dtype=np.int64)

    deg = np.bincount(dst, minlength=N).astype(np.int64) + 1
    dinv = (1.0 / np.sqrt(deg)).astype(np.float32)

    # snake assignment by degree rank balances per-(core,tile) edge loads
    order = np.argsort(-deg, kind="stable")
    rank = np.empty(N, dtype=np.int64)
    rank[order] = np.arange(N)
    row = rank // NCORES
    fwd = (row % 2) == 0
    core_of = np.where(fwd, rank % NCORES, NCORES - 1 - rank % NCORES)
    slot_of = row
    raw_of = core_of * SLOTS + slot_of
    grow_of = raw_of + (raw_of >= LOB)      # table row per node

    # self loops are NOT materialized as edges: each tile's self contribution
    # is added at consume time from the core's own (contiguous) table slab
    es = src
    ed = dst
    ecore = core_of[ed]
    eslot = slot_of[ed]
    etile = eslot // P
    edstloc = eslot % P
    esg = grow_of[es]
    islo = esg < LOB

    # per-core sorted segment arrays
    NSEG = TILES * 2   # segment id: 2*tile + (0 lo / 1 hi)
    per_core = []
    seg_counts = np.zeros((NCORES, NSEG), dtype=np.int64)
    for c in range(NCORES):
        m = ecore == c
        seg = etile[m] * 2 + (~islo[m]).astype(np.int64)
        o2 = np.lexsort((esg[m], seg))
        d = {
            "seg": seg[o2],
            "dstloc": edstloc[m][o2],
            "esg": esg[m][o2],
        }
        seg_counts[c] = np.bincount(d["seg"], minlength=NSEG)
        per_core.append(d)

    # chunk plan: per tile, lo/hi chunk counts = max over cores
    CLO = np.maximum(1, np.ceil(seg_counts[:, 0::2].max(axis=0) / P)).astype(int)
    CHI = np.maximum(1, np.ceil(seg_counts[:, 1::2].max(axis=0) / P)).astype(int)
    # chunk order: group-major; within group: all lo chunks (tile order), then hi
    seg_chunk_start = np.zeros(NSEG, dtype=np.int64)   # global chunk idx per seg
    grp_clo = np.zeros(NGROUPS, dtype=np.int64)
    grp_chi = np.zeros(NGROUPS, dtype=np.int64)
    gcb = np.zeros(NGROUPS + 1, dtype=np.int64)
    for g in range(NGROUPS):
        ts = range(GROUP_T0[g], GROUP_T0[g] + GROUP_SIZES[g])
        grp_clo[g] = sum(CLO[t] for t in ts)
        grp_chi[g] = sum(CHI[t] for t in ts)
        ofs = gcb[g]
        for t in ts:
            seg_chunk_start[2 * t] = ofs
            ofs += CLO[t]
        for t in ts:
            seg_chunk_start[2 * t + 1] = ofs
            ofs += CHI[t]
        gcb[g + 1] = ofs
    CTOT = int(gcb[-1])
    CPAD = (CTOT + KB - 1) // KB * KB

    # per-chunk default fill (pads): lo chunks -> ZLO, hi chunks -> absolute hi zero
    chunk_is_hi = np.zeros(CTOT, dtype=bool)
    for t in range(TILES):
        s = seg_chunk_start[2 * t + 1]
        chunk_is_hi[s:s + CHI[t]] = True

    pl.cores = []
    for c in range(NCORES):
        d = per_core[c]
        npad = CTOT * P
        dstloc_pad = np.zeros(npad, dtype=np.float16)
        row_pad = np.where(np.repeat(chunk_is_hi, P), TAB - 1, ZLO).astype(np.int64)
        # position of each real edge
        cnt = seg_counts[c]
        seg_first = np.concatenate([[0], np.cumsum(cnt)[:-1]])
        within = np.arange(len(d["seg"])) - seg_first[d["seg"]]
        pos = seg_chunk_start[d["seg"]] * P + within
        dstloc_pad[pos] = d["dstloc"].astype(np.float16)
        row_pad[pos] = d["esg"]

        # gather index arrays (lo then hi, group-major)
        lo_parts, hi_parts = [], []
        for g in range(NGROUPS):
            a = gcb[g] * P
            b = a + grp_clo[g] * P
            e = gcb[g + 1] * P
            lo_parts.append(row_pad[a:b])
            hi_parts.append(row_pad[b:e] - 32768)
        lo_flat = np.concatenate(lo_parts)
        hi_flat = np.concatenate(hi_parts)
        assert lo_flat.min() >= 0 and lo_flat.max() <= LOB
        assert hi_flat.min() >= 0 and hi_flat.max() <= ZHI_LOCAL

        dl = np.zeros((P, CPAD), dtype=np.float16)
        dl[:, :CTOT] = dstloc_pad.reshape(CTOT, P).T
        core = {
            "idxlo": _wrap_idx(lo_flat),
            "idxhi": _wrap_idx(hi_flat),
            "dstloc": dl.copy(),
        }
        pl.cores.append(core)

    # group gather call metadata (columns into wrapped idx tensors)
    pl.lo_cols = int(grp_clo.sum() * P // 16)
    pl.hi_cols = int(grp_chi.sum() * P // 16)
    lo_c0 = np.concatenate([[0], np.cumsum(grp_clo * 8)])
    hi_c0 = np.concatenate([[0], np.cumsum(grp_chi * 8)])
    pl.groups = []
    for g in range(NGROUPS):
        tiles = []
        for t in range(GROUP_T0[g], GROUP_T0[g] + GROUP_SIZES[g]):
            lo_local = seg_chunk_start[2 * t] - gcb[g]
            hi_local = seg_chunk_start[2 * t + 1] - gcb[g]
            chunks = [(int(lo_local + j), int(seg_chunk_start[2 * t] + j))
                      for j in range(CLO[t])]
            chunks += [(int(hi_local + j), int(seg_chunk_start[2 * t + 1] + j))
                       for j in range(CHI[t])]
            tiles.append(chunks)
        pl.groups.append({
            "nclo": int(grp_clo[g]), "nchi": int(grp_chi[g]),
            "lo_col0": int(lo_c0[g]), "hi_col0": int(hi_c0[g]),
            "tiles": tiles,
        })
    pl.CTOT = CTOT
    pl.CPAD = CPAD

    # slot -> node map, dinv per slot, pooling matrix, xT shards, row map
    node_at = np.full((NCORES, SLOTS), -1, dtype=np.int64)
    node_at[core_of, slot_of] = np.arange(N)
    valid = node_at >= 0
    counts = np.bincount(batch, minlength=NGRAPH).astype(np.float64)
    dinv_slot = np.ones((NCORES, SLOTS), dtype=np.float32)
    dinv_slot[valid] = dinv[node_at[valid]]
    xs = np.asarray(x, dtype=np.float32) * dinv[:, None]   # fold dinv_src for L1
    for c in range(NCORES):
        cc = pl.cores[c]
        cc["dinvslot"] = dinv_slot[c].reshape(TILES, P).T.astype(np.float32).copy()
        pm = np.zeros((SLOTS, NGRAPH), dtype=np.float32)
        v = valid[c]
        nd = node_at[c][v]
        pm[v, batch[nd]] = dinv[nd] / counts[batch[nd]]
        # [P, TILES, NGRAPH]
        cc["poolmat"] = pm.reshape(TILES, P, NGRAPH).transpose(1, 0, 2).astype(
            np.float16).copy()
        xt = np.zeros((D_IN, SLOTS), dtype=np.float16)
        xt[:, v] = xs[nd].T.astype(np.float16)
        cc["xT"] = xt

    rm = np.arange(RAW, dtype=np.int64)
    pl.rowmap = (rm + (rm >= LOB)).reshape(NCORES, SLOTS)
    pl.counts = counts
    pl.iotarep = np.arange(P, dtype=np.float16).repeat(KB).reshape(1, P * KB)
    pl.key = (tuple(CLO), tuple(CHI))
    return pl


# ---------------------------------------------------------- program builders --

def _new_bacc():
    return bacc.Bacc("TRN2", target_bir_lowering=False, debug=False,
                     num_devices=NCORES)


def _build_A():
    nc = _new_bacc()
    i_xT = nc.dram_tensor("xT", [D_IN, SLOTS], F16, kind="ExternalInput").ap()
    i_W = nc.dram_tensor("W", [D_IN, H], F16, kind="ExternalInput").ap()
    o_T = nc.dram_tensor("Tout", [SLOTS, H], F16, kind="ExternalOutput").ap()
    with tile.TileContext(nc) as tc:
        with ExitStack() as ctx:
            const = ctx.enter_context(tc.tile_pool(name="const", bufs=1))
            gp_pool = ctx.enter_context(tc.tile_pool(name="gp", bufs=2, space="PSUM"))
            ot_pool = ctx.enter_context(tc.tile_pool(name="ot", bufs=3))
            w0 = const.tile([P, H], F16)
            nc.sync.dma_start(out=w0[:], in_=i_W[0:P, :])
            w1 = const.tile([P, H], F16)
            nc.sync.dma_start(out=w1[:], in_=i_W[P:2 * P, :])
            # load x in 7 column-slabs so matmuls start early
            XCH = SLOTS // 7
            x0 = const.tile([P, SLOTS], F16)
            x1 = const.tile([P, SLOTS], F16)
            for o in range(0, SLOTS, XCH):
                nc.sync.dma_start(out=x0[:, o:o + XCH], in_=i_xT[0:P, o:o + XCH])
                nc.sync.dma_start(out=x1[:, o:o + XCH],
                                  in_=i_xT[P:2 * P, o:o + XCH])
            OB = 7  # tiles per output DMA
            for t in range(TILES):
                gp = gp_pool.tile([P, H], F32, space="PSUM")
                nc.tensor.matmul(out=gp[:], lhsT=x0[:, t * P:(t + 1) * P],
                                 rhs=w0[:], start=True, stop=False)
                nc.tensor.matmul(out=gp[:], lhsT=x1[:, t * P:(t + 1) * P],
                                 rhs=w1[:], start=False, stop=True)
                if t % OB == 0:
                    ob = ot_pool.tile([P, OB, H], F16, tag="ob", name="ob")
                nc.scalar.activation(out=ob[:, t % OB, :], in_=gp[:],
                                     func=mybir.ActivationFunctionType.Copy,
                                     scale=1.0)
                if t % OB == OB - 1:
                    t0 = t - OB + 1
                    nc.sync.dma_start(
                        out=o_T[t0 * P:(t + 1) * P, :].rearrange(
                            "(b p) h -> p b h", p=P),
                        in_=ob[:])
    nc.compile()
    return nc


def _scatter_body(nc, ctx, tc, pl, i_T, consume_tile):
    """Shared gather + batched-0/1-one-hot matmul scatter loop.

    consume_tile(t, ypsum) handles the per-tile PSUM result [128 dst, H].
    """
    const = ctx.enter_context(tc.tile_pool(name="sc_const", bufs=1))
    stage = ctx.enter_context(tc.tile_pool(name="staging", bufs=2))
    oh_pool = ctx.enter_context(tc.tile_pool(name="oh", bufs=4))
    yp_pool = ctx.enter_context(tc.tile_pool(name="yps", bufs=3, space="PSUM"))

    i_idxlo = nc.dram_tensor("idxlo", [P, pl.lo_cols], I16, kind="ExternalInput").ap()
    i_idxhi = nc.dram_tensor("idxhi", [P, pl.hi_cols], I16, kind="ExternalInput").ap()
    i_dstloc = nc.dram_tensor("dstloc", [P, pl.CPAD], F16, kind="ExternalInput").ap()
    i_iota = nc.dram_tensor("iotarep", [1, P * KB], F16, kind="ExternalInput").ap()
    i_self = nc.dram_tensor("selfT", [P, TILES, H], F16, kind="ExternalInput").ap()

    # per-group index slices so the first gather starts ASAP
    idxlo_sb = const.tile([P, pl.lo_cols], I16)
    idxhi_sb = const.tile([P, pl.hi_cols], I16)
    for g, grp in enumerate(pl.groups):
        a, n = grp["lo_col0"], grp["nclo"] * 8
        nc.sync.dma_start(out=idxlo_sb[:, a:a + n], in_=i_idxlo[:, a:a + n])
        a, n = grp["hi_col0"], grp["nchi"] * 8
        nc.sync.dma_start(out=idxhi_sb[:, a:a + n], in_=i_idxhi[:, a:a + n])
    dstloc_sb = const.tile([P, pl.CPAD], F16)
    nc.sync.dma_start(out=dstloc_sb[:], in_=i_dstloc[:])
    iota_sb = const.tile([P, P * KB], F16)
    nc.sync.dma_start(out=iota_sb[:], in_=i_iota.to_broadcast([P, P * KB]))
    self_sb = const.tile([P, TILES, H], F16)
    nc.sync.dma_start(out=self_sb[:], in_=i_self[:])

    oh_views = {}

    def get_oh(gc):
        r = gc // KB
        if r not in oh_views:
            oh = oh_pool.tile([P, P * KB], F16, tag="oh", name="oh")
            bc = dstloc_sb[:, r * KB:(r + 1) * KB].rearrange(
                "p (o k) -> p o k", o=1).broadcast_to([P, P, KB])
            nc.vector.tensor_tensor(
                out=oh[:].rearrange("p (j k) -> p j k", k=KB),
                in0=iota_sb[:].rearrange("p (j k) -> p j k", k=KB),
                in1=bc, op=mybir.AluOpType.is_equal)
            oh_views[r] = oh[:].rearrange("p (j k) -> p k j", k=KB)
        return oh_views[r][:, gc % KB, :]

    for g, grp in enumerate(pl.groups):
        nclo, nchi = grp["nclo"], grp["nchi"]
        staging = stage.tile([P, nclo + nchi, H], F16, tag="staging")
        for o in range(0, nclo, MAXCH):
            n = min(MAXCH, nclo - o)
            c0 = grp["lo_col0"] + o * 8
            nc.gpsimd.dma_gather(
                out_ap=staging[:, o:o + n, :], in_ap=i_T[:],
                idxs_ap=idxlo_sb[:, c0:c0 + n * 8],
                num_idxs=n * P, num_idxs_reg=n * P, elem_size=H)
        for o in range(0, nchi, MAXCH):
            n = min(MAXCH, nchi - o)
            c0 = grp["hi_col0"] + o * 8
            nc.gpsimd.dma_gather(
                out_ap=staging[:, nclo + o:nclo + o + n, :], in_ap=i_T[32768:, :],
                idxs_ap=idxhi_sb[:, c0:c0 + n * 8],
                num_idxs=n * P, num_idxs_reg=n * P, elem_size=H)
        for ti, chunks in enumerate(grp["tiles"]):
            t = GROUP_T0[g] + ti
            ypsum = yp_pool.tile([P, H], F32, space="PSUM")
            for j, (sp, gc) in enumerate(chunks):
                nc.tensor.matmul(out=ypsum[:], lhsT=get_oh(gc),
                                 rhs=staging[:, sp, :],
                                 start=(j == 0), stop=(j == len(chunks) - 1))
            consume_tile(t, ypsum, self_sb)


def _build_BC(pl):
    nc = _new_bacc()
    i_T = nc.dram_tensor("T", [TAB, H], F16, kind="ExternalInput").ap()
    i_W = nc.dram_tensor("W", [H, H], F16, kind="ExternalInput").ap()
    i_scale = nc.dram_tensor("bnscale", [H, 1], F32, kind="ExternalInput").ap()
    i_bias = nc.dram_tensor("bnbias", [H, 1], F32, kind="ExternalInput").ap()
    i_dinv = nc.dram_tensor("dinvslot", [P, TILES], F32, kind="ExternalInput").ap()
    o_T = nc.dram_tensor("Tout", [SLOTS, H], F16, kind="ExternalOutput").ap()
    with tile.TileContext(nc) as tc:
        with ExitStack() as ctx:
            const = ctx.enter_context(tc.tile_pool(name="bc_const", bufs=1))
            ycp_pool = ctx.enter_context(tc.tile_pool(name="ycp", bufs=3))
            tps_pool = ctx.enter_context(tc.tile_pool(name="tps", bufs=2, space="PSUM"))
            ht_pool = ctx.enter_context(tc.tile_pool(name="ht", bufs=3))
            gp_pool = ctx.enter_context(tc.tile_pool(name="gp", bufs=2, space="PSUM"))
            ot_pool = ctx.enter_context(tc.tile_pool(name="ot", bufs=3))

            scale_sb = const.tile([H, 1], F32)
            nc.sync.dma_start(out=scale_sb[:], in_=i_scale[:])
            bias_sb = const.tile([H, 1], F32)
            nc.sync.dma_start(out=bias_sb[:], in_=i_bias[:])
            dinv_sb = const.tile([P, TILES], F32)
            nc.sync.dma_start(out=dinv_sb[:], in_=i_dinv[:])
            w_sb = const.tile([H, H], F16)
            nc.sync.dma_start(out=w_sb[:], in_=i_W[:])
            ident16 = const.tile([P, P], F16)
            make_identity(nc, ident16[:])
            obuf = {}

            def consume(t, ypsum, self_sb):
                # ycp = dinv_dst * Y_gathered + self_msg (selfT host-prescaled)
                ycp = ycp_pool.tile([P, H], F16)
                nc.vector.scalar_tensor_tensor(
                    out=ycp[:], in0=ypsum[:], scalar=dinv_sb[:, t:t + 1],
                    in1=self_sb[:, t, :],
                    op0=mybir.AluOpType.mult, op1=mybir.AluOpType.add)
                tp = tps_pool.tile([P, P], F16, space="PSUM")
                nc.tensor.transpose(out=tp[:], in_=ycp[:], identity=ident16[:])
                ht = ht_pool.tile([P, P], F16)
                nc.scalar.activation(out=ht[:], in_=tp[:],
                                     func=mybir.ActivationFunctionType.Relu,
                                     bias=bias_sb[:], scale=scale_sb[:])
                gp = gp_pool.tile([P, H], F32, space="PSUM")
                nc.tensor.matmul(out=gp[:], lhsT=ht[:], rhs=w_sb[:],
                                 start=True, stop=True)
                OB = 7
                if t % OB == 0:
                    obuf["t"] = ot_pool.tile([P, OB, H], F16, tag="ob", name="ob")
                ob = obuf["t"]
                nc.scalar.activation(out=ob[:, t % OB, :], in_=gp[:],
                                     func=mybir.ActivationFunctionType.Copy,
                                     scale=dinv_sb[:, t:t + 1])
                if t % OB == OB - 1:
                    t0 = t - OB + 1
                    nc.sync.dma_start(
                        out=o_T[t0 * P:(t + 1) * P, :].rearrange(
                            "(b p) h -> p b h", p=P),
                        in_=ob[:])

            _scatter_body(nc, ctx, tc, pl, i_T, consume)
    nc.compile()
    return nc


def _build_D(pl):
    nc = _new_bacc()
    i_T = nc.dram_tensor("T", [TAB, H], F16, kind="ExternalInput").ap()
    i_pm = nc.dram_tensor("poolmat", [P, TILES, NGRAPH], F16,
                          kind="ExternalInput").ap()
    o_pool = nc.dram_tensor("pool", [NGRAPH, H], F32, kind="ExternalOutput").ap()
    with tile.TileContext(nc) as tc:
        with ExitStack() as ctx:
            const = ctx.enter_context(tc.tile_pool(name="d_const", bufs=1))
            h3_pool = ctx.enter_context(tc.tile_pool(name="h3", bufs=3))
            pp_pool = ctx.enter_context(tc.tile_pool(name="pp", bufs=1, space="PSUM"))

            pm_sb = const.tile([P, TILES, NGRAPH], F16)
            nc.sync.dma_start(out=pm_sb[:], in_=i_pm[:])
            pp = pp_pool.tile([NGRAPH, H], F32, space="PSUM")

            def consume(t, ypsum, self_sb):
                h3 = h3_pool.tile([P, H], F16)
                nc.vector.tensor_tensor(out=h3[:], in0=ypsum[:],
                                        in1=self_sb[:, t, :],
                                        op=mybir.AluOpType.add)
                nc.tensor.matmul(out=pp[:], lhsT=pm_sb[:, t, :], rhs=h3[:],
                                 start=(t == 0), stop=(t == TILES - 1))

            _scatter_body(nc, ctx, tc, pl, i_T, consume)
            pool_sb = const.tile([NGRAPH, H], F32)
            nc.vector.tensor_copy(out=pool_sb[:], in_=pp[:])
            nc.sync.dma_start(out=o_pool[:], in_=pool_sb[:])
    nc.compile()
    return nc


# ------------------------------------------------------------------- driver --

def _run(nc, in_maps):
    res = run_bass_kernel_spmd(nc, in_maps, core_ids=list(range(NCORES)),
                               trace=TRACE)
    if TRACE:
        LAST_EXEC_NS.append(res.exec_time_ns)
    return res.results


def _assemble_table(pl, shards):
    T = np.zeros((TAB, H), dtype=np.float16)
    for c in range(NCORES):
        T[pl.rowmap[c]] = shards[c]
    return T


def _self_arr(shard, dinvslot=None):
    arr = np.asarray(shard, dtype=np.float32).reshape(TILES, P, H).transpose(1, 0, 2)
    if dinvslot is not None:
        arr = arr * dinvslot[:, :, None]
    return arr.astype(np.float16)


def _bn_fold(b, g, beta, m, v):
    s = (g.astype(np.float64) / np.sqrt(v.astype(np.float64) + BN_EPS))
    bias = (b.astype(np.float64) - m.astype(np.float64)) * s + beta.astype(np.float64)
    return (s.astype(np.float32).reshape(H, 1),
            bias.astype(np.float32).reshape(H, 1))


def kernel(**inputs):
    ins = {k: np.asarray(v) for k, v in inputs.items()}
    key = hashlib.sha1(
        ins["edge_index"].tobytes() + ins["batch"].tobytes()
        + ins["x"].tobytes()
    ).hexdigest()
    if key not in _PLAN_CACHE:
        _PLAN_CACHE[key] = _make_plan(ins["edge_index"], ins["batch"], ins["x"])
    pl = _PLAN_CACHE[key]

    pk = pl.key
    if pk not in _PROG_CACHE:
        _PROG_CACHE[pk] = {
            "A": _build_A(),
            "BC": _build_BC(pl),
            "D": _build_D(pl),
        }
    progs = _PROG_CACHE[pk]

    LAST_EXEC_NS.clear()
    W1 = ins["W1"].astype(np.float16)
    # Launch A: T1 = (dinv*x) @ W1
    resA = _run(progs["A"], [
        {"xT": pl.cores[c]["xT"], "W": W1} for c in range(NCORES)
    ])
    T1 = _assemble_table(pl, [r["Tout"] for r in resA])

    def meta(c):
        cc = pl.cores[c]
        return {"idxlo": cc["idxlo"], "idxhi": cc["idxhi"],
                "dstloc": cc["dstloc"], "iotarep": pl.iotarep}

    shardA = [r["Tout"] for r in resA]
    s1, b1f = _bn_fold(ins["b1"], ins["bn1_g"], ins["bn1_b"],
                       ins["bn1_m"], ins["bn1_v"])
    # Launch B: layer-1 scatter + BN1/ReLU + @W2
    resB = _run(progs["BC"], [
        {**meta(c), "T": T1, "W": ins["W2"].astype(np.float16),
         "bnscale": s1, "bnbias": b1f, "dinvslot": pl.cores[c]["dinvslot"],
         "selfT": _self_arr(shardA[c], pl.cores[c]["dinvslot"])}
        for c in range(NCORES)
    ])
    shardB = [r["Tout"] for r in resB]
    T2 = _assemble_table(pl, shardB)

    s2, b2f = _bn_fold(ins["b2"], ins["bn2_g"], ins["bn2_b"],
                       ins["bn2_m"], ins["bn2_v"])
    # Launch C: layer-2 scatter + BN2/ReLU + @W3
    resC = _run(progs["BC"], [
        {**meta(c), "T": T2, "W": ins["W3"].astype(np.float16),
         "bnscale": s2, "bnbias": b2f, "dinvslot": pl.cores[c]["dinvslot"],
         "selfT": _self_arr(shardB[c], pl.cores[c]["dinvslot"])}
        for c in range(NCORES)
    ])
    shardC = [r["Tout"] for r in resC]
    T3 = _assemble_table(pl, shardC)

    # Launch D: layer-3 scatter + pooled means (dinv_dst and 1/counts folded
    # into poolmat on the host; self contribution unscaled)
    resD = _run(progs["D"], [
        {**meta(c), "T": T3, "poolmat": pl.cores[c]["poolmat"],
         "selfT": _self_arr(shardC[c])}
        for c in range(NCORES)
    ])
    pooled = np.sum([r["pool"] for r in resD], axis=0).astype(np.float64)
    pooled += ins["b3"].astype(np.float64)[None, :]

    z = np.maximum(pooled @ ins["Wc1"].astype(np.float64)
                   + ins["bc1"].astype(np.float64), 0.0)
    out = z @ ins["Wc2"].astype(np.float64) + ins["bc2"].astype(np.float64)
    return out.astype(np.float32)
